# revision 6
# baseline (speedup 1.0000x reference)
"""Trainium2 Bass kernel for nn_GPSODModel (GraphGPS: GINEConv + global MHA + FFN +
GraphNorm x3, bilinear O-D pair decoder).

Self-contained: takes FULL inputs, shards across 8 NeuronCores internally
(nodes + their incident edges by destination), runs one SPMD Bass program with
one AllGather per layer, returns the FULL [N, N] output.

Layout convention on-chip: activations are kept feature-major ("xT": features on
the 128 SBUF partitions, nodes along the free axis) so every linear layer is a
single chain of matmuls with host-pre-transposed weights.  Node-major copies
(h_nm) live in DRAM for the per-edge source gather (dma_gather).
"""

import sys

sys.path.insert(0, "/opt/trn_rl_repo")

import numpy as np
import ml_dtypes

import concourse.bacc as bacc
import concourse.bass as bass
import concourse.mybir as mybir
import concourse.tile as tile
from concourse.masks import make_identity
from concourse.bass_utils import run_bass_kernel_spmd

# ---------------------------------------------------------------- constants
N, E = 4096, 131072
IDIM, ED = 64, 32
HD, L, NH = 128, 3, 4
DH = HD // NH
SCALE = 1.0 / float(np.sqrt(DH))

NCORES = 8
NL = N // NCORES            # own nodes per core (512)
NWIN = NL // 128            # dst windows per core (4)
P = 128
GROUP = 1024                # edges per gather/stream group (8 chunks of 128)

f32 = mybir.dt.float32
bf16 = mybir.dt.bfloat16
i16 = mybir.dt.int16

AF = mybir.ActivationFunctionType
OP = mybir.AluOpType

LAST_RESULTS = None         # test harness reads timing info from here


# ---------------------------------------------------------------- host prep
class _WPack:
    """Packs host weight matrices into one [128, WC] f32 blob."""

    def __init__(self):
        self.cols = 0
        self.entries = {}       # name -> (rows, c0, ncols)
        self.mats = {}

    def add(self, name, mat):
        mat = np.asarray(mat, np.float32)
        assert mat.ndim == 2 and mat.shape[0] <= P, (name, mat.shape)
        r, c = mat.shape
        self.entries[name] = (r, self.cols, c)
        self.mats[name] = mat
        self.cols += c

    def blob(self):
        out = np.zeros((P, self.cols), np.float32)
        for name, (r, c0, c) in self.entries.items():
            out[:r, c0:c0 + c] = self.mats[name]
        return out


def _pack_weights(p):
    w = _WPack()
    T = lambda m: np.ascontiguousarray(np.asarray(m, np.float32).T)
    w.add("npW1T", T(p["np_W1"]))            # [64, 128]
    w.add("npW2T", T(p["np_W2"]))            # [128, 128]
    w.add("epW1T", T(p["ep_W1"]))            # [32, 128]
    w.add("epW2T", T(p["ep_W2"]))            # [128, 128]
    for i in range(L):
        w.add(f"gW1T{i}", T(p["gine_W1"][i]))
        w.add(f"gW2T{i}", T(p["gine_W2"][i]))
        qkv = np.asarray(p["attn_Wqkv"][i], np.float32)
        w.add(f"WqT{i}", T(qkv[0:HD]) * SCALE)
        w.add(f"WkT{i}", T(qkv[HD:2 * HD]))
        w.add(f"WvT{i}", T(qkv[2 * HD:3 * HD]))
        w.add(f"WoT{i}", T(p["attn_Wo"][i]))
        w.add(f"mW1T{i}", T(p["mlp_W1"][i]))         # [128, 256]
        m2t = T(p["mlp_W2"][i])                      # [256, 128]
        w.add(f"mW2Ta{i}", m2t[0:P])
        w.add(f"mW2Tb{i}", m2t[P:2 * P])
        w.add(f"gnG{i}", np.asarray(p["gn_gamma"][i], np.float32)[None, :])
        w.add(f"gnB{i}", np.asarray(p["gn_beta"][i], np.float32)[None, :])
    w.add("decWT", T(p["dec_W"]))
    return w


def _wrap_idx(idx, cols):
    """dma_gather index layout: elem i read from [i%16, i//16]; replicate to
    all 128 partitions."""
    n = idx.shape[0]
    assert n % 16 == 0
    a = np.zeros((16, cols), np.int16)
    a[:, : n // 16] = np.asarray(idx, np.int16).reshape(-1, 16).T
    return np.tile(a, (8, 1))


def _prep(x, edge_attr, params, edge_index):
    x = np.asarray(x, np.float32)
    edge_attr = np.asarray(edge_attr, np.float32)
    edge_index = np.asarray(edge_index)
    src, dst = edge_index[0].astype(np.int64), edge_index[1].astype(np.int64)

    for k in ("np_b1", "np_b2", "ep_b1", "ep_b2", "gine_b1", "gine_b2",
              "attn_bqkv", "attn_bo", "mlp_b1", "mlp_b2"):
        assert np.abs(np.asarray(params[k])).max() == 0.0, \
            f"nonzero bias {k} unsupported"

    # --- edge sharding: core = dst // NL, window = (dst % NL) // 128
    order = np.lexsort((src, dst))
    ds, ss = dst[order], src[order]
    win_of = ds // 128                       # global window id 0..31
    counts = np.bincount(win_of, minlength=N // 128)
    cpw = counts.reshape(NCORES, NWIN)       # [core, window] edge counts
    M = [int(np.ceil(cpw[:, w].max() / 128)) for w in range(NWIN)]
    g = GROUP // 128
    M[NWIN - 1] += (-sum(M)) % g
    E_pad = 128 * sum(M)

    chunk_window = []
    for w in range(NWIN):
        chunk_window += [w] * M[w]

    wp = _pack_weights(params)
    wblob = wp.blob()
    xT = np.ascontiguousarray(x.T)                      # [64, 4096]

    starts = np.zeros(N // 128 + 1, np.int64)
    starts[1:] = np.cumsum(counts)
    woff = np.cumsum([0] + M)                            # chunk offset per window

    in_maps = []
    for c in range(NCORES):
        srcs = np.zeros(E_pad, np.int64)
        dloc = np.full(E_pad, -1, np.int64)             # window-local dst or -1
        ea = np.zeros((E_pad, ED), np.float32)
        for w in range(NWIN):
            gw = c * NWIN + w
            s, e = starts[gw], starts[gw + 1]
            k = e - s
            off = 128 * woff[w]
            srcs[off:off + k] = ss[s:e]
            dloc[off:off + k] = ds[s:e] - c * NL - w * 128
            ea[off:off + k] = edge_attr[order[s:e]]
        oh = np.zeros((E_pad, P), np.float32)
        valid = dloc >= 0
        oh[np.arange(E_pad)[valid], dloc[valid]] = 1.0
        # SBUF layout [p, chunk*128 + d] = oh[chunk*128 + p, d]
        ohs = np.ascontiguousarray(
            oh.reshape(E_pad // P, P, P).transpose(1, 0, 2).reshape(P, E_pad)
        ).astype(np.float32)
        eaT = np.ascontiguousarray(ea.T)                # [32, E_pad]
        srcidx = _wrap_idx(srcs, E_pad // 16)
        ownidx = _wrap_idx(np.arange(c * NL, (c + 1) * NL), NL // 16)
        in_maps.append({
            "xT": xT,
            "eaT": eaT,
            "oh": ohs,
            "srcidx": srcidx,
            "ownidx": ownidx,
            "wblob": wblob,
        })
    meta = {"E_pad": E_pad, "M": M, "chunk_window": chunk_window, "wp": wp,
            "alpha": [float(a) for a in np.asarray(params["gn_alpha"], np.float32)]}
    return in_maps, meta


# ---------------------------------------------------------------- program
def _build(meta):
    E_pad = meta["E_pad"]
    cw = meta["chunk_window"]
    ngroups = E_pad // GROUP
    gpc = GROUP // 128            # chunks per group

    nc = bacc.Bacc(None)

    # --- dram tensors
    xT_d = nc.dram_tensor("xT", [IDIM, N], f32, kind="ExternalInput")
    eaT_d = nc.dram_tensor("eaT", [ED, E_pad], f32, kind="ExternalInput")
    oh_d = nc.dram_tensor("oh", [P, E_pad], f32, kind="ExternalInput")
    srcidx_d = nc.dram_tensor("srcidx", [P, E_pad // 16], i16, kind="ExternalInput")
    ownidx_d = nc.dram_tensor("ownidx", [P, NL // 16], i16, kind="ExternalInput")
    wblob_d = nc.dram_tensor("wblob", [P, meta["wp"].cols], f32, kind="ExternalInput")
    od_d = nc.dram_tensor("od", [NL, N], f32, kind="ExternalOutput")

    hnm_d = [nc.dram_tensor(f"hnm{i}", [N, HD], f32) for i in range(L + 1)]
    ee_d = nc.dram_tensor("ee", [E_pad, HD], f32)
    cc_in = [nc.dram_tensor(f"ccin{i}", [NL, HD], f32) for i in range(L)]
    cc_out = [nc.dram_tensor(f"ccout{i}", [N, HD], f32, addr_space="Shared")
              for i in range(L)]

    with tile.TileContext(nc) as tc:
        wpool = tc.alloc_tile_pool(name="wpool", bufs=1)
        hpool = tc.alloc_tile_pool(name="hpool", bufs=2)
        state = tc.alloc_tile_pool(name="state", bufs=1)
        stream = tc.alloc_tile_pool(name="stream", bufs=2)
        big = tc.alloc_tile_pool(name="big", bufs=2)
        tp = tc.alloc_tile_pool(name="tp", bufs=4)
        rows = tc.alloc_tile_pool(name="rows", bufs=2)
        psA = tc.alloc_tile_pool(name="psA", bufs=1, space="PSUM")
        psB = tc.alloc_tile_pool(name="psB", bufs=2, space="PSUM")
        psC = tc.alloc_tile_pool(name="psC", bufs=1, space="PSUM")

        # --- constants
        wblob = wpool.tile([P, meta["wp"].cols], f32)
        nc.sync.dma_start(wblob[:], wblob_d[:])

        def W(name):
            r, c0, ncol = meta["wp"].entries[name]
            return wblob[:r, c0:c0 + ncol]

        ident = wpool.tile([P, P], f32)
        make_identity(nc, ident[:])
        ones32 = wpool.tile([P, 32], f32)
        nc.vector.memset(ones32[:], 1.0)
        ones_row = wpool.tile([1, P], f32)
        nc.vector.memset(ones_row[:], 1.0)

        srcidx_s = wpool.tile([P, E_pad // 16], i16)
        nc.sync.dma_start(srcidx_s[:], srcidx_d[:])
        ownidx_s = wpool.tile([P, NL // 16], i16)
        nc.sync.dma_start(ownidx_s[:], ownidx_d[:])

        def trans128(dst_ap, src_ap):
            """dst[c, n] = src[n, c] via PE transpose (both [128, 128])."""
            ps = psB.tile([P, 512], f32, tag="ps")
            nc.tensor.transpose(ps[:, :P], src_ap, ident[:])
            nc.vector.tensor_copy(dst_ap, ps[:, :P])

        # ---------------- prologue: h0 = MLP(x), both layouts -------------
        hT = hpool.tile([P, N], f32, tag="hT")
        for t in range(N // 512):
            ps = psB.tile([P, 512], f32, tag="ps")
            xs = stream.tile([IDIM, 512], f32, tag="xs")
            nc.sync.dma_start(xs[:], xT_d[:, 512 * t:512 * (t + 1)])
            nc.tensor.matmul(ps[:], W("npW1T"), xs[:], start=True, stop=True)
            mid = stream.tile([P, 512], f32, tag="mid")
            nc.vector.tensor_scalar_max(mid[:], ps[:], 0.0)
            ps2 = psB.tile([P, 512], f32, tag="ps")
            nc.tensor.matmul(ps2[:], W("npW2T"), mid[:], start=True, stop=True)
            nc.vector.tensor_copy(hT[:, 512 * t:512 * (t + 1)], ps2[:])
            # node-major h0 -> DRAM (for edge gather), 4 tiles of 128
            for k in range(4):
                t128 = tp.tile([P, P], f32, tag="tp")
                trans128(t128[:], hT[:, 512 * t + P * k: 512 * t + P * (k + 1)])
                nc.sync.dma_start(
                    hnm_d[0][512 * t + P * k: 512 * t + P * (k + 1), :], t128[:])

        # ---------------- prologue: ee = MLP(edge_attr) -> DRAM edge-major
        for g in range(E_pad // 512):
            eas = stream.tile([ED, 512], f32, tag="eas")
            nc.sync.dma_start(eas[:], eaT_d[:, 512 * g:512 * (g + 1)])
            ps = psB.tile([P, 512], f32, tag="ps")
            nc.tensor.matmul(ps[:], W("epW1T"), eas[:], start=True, stop=True)
            mid = stream.tile([P, 512], f32, tag="mid")
            nc.vector.tensor_scalar_max(mid[:], ps[:], 0.0)
            # edge-major out: lhsT = mid chunk [c, e], rhs = epW2T [c, c2]
            for k in range(4):
                ps2 = psB.tile([P, 512], f32, tag="ps")
                nc.tensor.matmul(ps2[:, :P], mid[:, P * k:P * (k + 1)],
                                 W("epW2T"), start=True, stop=True)
                t128 = tp.tile([P, P], f32, tag="tp")
                nc.vector.tensor_copy(t128[:], ps2[:, :P])
                nc.sync.dma_start(
                    ee_d[512 * g + P * k: 512 * g + P * (k + 1), :], t128[:])

        # ---------------- layers ------------------------------------------
        for li in range(L):
            # own node rows (node-major) for this layer
            hnm_own = state.tile([P, NWIN, P], f32, tag="hnm_own")
            nc.gpsimd.dma_gather(
                out_ap=hnm_own[:], in_ap=hnm_d[li][:], idxs_ap=ownidx_s[:],
                num_idxs=NL, num_idxs_reg=NL, elem_size=HD)
            hTown = state.tile([P, NL], f32, tag="hTown")
            for w in range(NWIN):
                trans128(hTown[:, P * w:P * (w + 1)], hnm_own[:, w, :])

            # ---- GINE: gather h[src], msg = relu(h_src + ee), scatter-add
            hgaT = state.tile([P, NL], f32, tag="hgaT")
            aggr_ps = None
            cur_w = -1
            left = {w: meta["M"][w] for w in range(NWIN)}
            for g in range(ngroups):
                gat = stream.tile([P, gpc, P], f32, tag="gat")
                nc.gpsimd.dma_gather(
                    out_ap=gat[:], in_ap=hnm_d[li][:],
                    idxs_ap=srcidx_s[:, (GROUP // 16) * g:(GROUP // 16) * (g + 1)],
                    num_idxs=GROUP, num_idxs_reg=GROUP, elem_size=HD)
                eet = stream.tile([P, gpc, P], f32, tag="eet")
                nc.sync.dma_start(
                    eet[:],
                    ee_d[GROUP * g:GROUP * (g + 1), :].rearrange(
                        "(k p) c -> p k c", p=P))
                oht = stream.tile([P, GROUP], f32, tag="oht")
                nc.sync.dma_start(oht[:], oh_d[:, GROUP * g:GROUP * (g + 1)])
                msgf = stream.tile([P, gpc, P], f32, tag="msgf")
                nc.vector.tensor_tensor(out=msgf[:], in0=gat[:], in1=eet[:],
                                        op=OP.add)
                msg = stream.tile([P, gpc, P], f32, tag="msg")
                nc.vector.tensor_scalar_max(msg[:], msgf[:], 0.0)
                for k in range(gpc):
                    chunk = g * gpc + k
                    w = cw[chunk]
                    if w != cur_w:
                        aggr_ps = psB.tile([P, 512], f32, tag="ps")
                        cur_w = w
                    first = left[w] == meta["M"][w]
                    left[w] -= 1
                    last = left[w] == 0
                    nc.tensor.matmul(
                        aggr_ps[:, :P],
                        oht[:, P * k:P * (k + 1)],
                        msg[:, k, :],
                        start=first, stop=last)
                    if last:
                        # h + aggr (node-major), transpose into hgaT
                        hga = tp.tile([P, P], f32, tag="tp")
                        nc.vector.tensor_tensor(
                            out=hga[:], in0=aggr_ps[:, :P],
                            in1=hnm_own[:, w, :], op=OP.add)
                        trans128(hgaT[:, P * w:P * (w + 1)], hga[:])

            # GINE MLP + residual
            ps = psB.tile([P, 512], f32, tag="ps")
            nc.tensor.matmul(ps[:], W(f"gW1T{li}"), hgaT[:], start=True, stop=True)
            gmid = state.tile([P, NL], f32, tag="gmid")
            nc.vector.tensor_scalar_max(gmid[:], ps[:], 0.0)
            ps2 = psB.tile([P, 512], f32, tag="ps")
            nc.tensor.matmul(ps2[:], W(f"gW2T{li}"), gmid[:], start=True, stop=True)
            h_loc = state.tile([P, NL], f32, tag="h_loc")
            nc.vector.tensor_tensor(out=h_loc[:], in0=ps2[:], in1=hTown[:], op=OP.add)

            # ---- attention: QT own, KT/V all nodes
            QT = state.tile([P, NL], f32, tag="QT")
            ps = psB.tile([P, 512], f32, tag="ps")
            nc.tensor.matmul(ps[:], W(f"WqT{li}"), hTown[:], start=True, stop=True)
            nc.vector.tensor_copy(QT[:], ps[:])
            KT = state.tile([P, N], f32, tag="KT")
            for t in range(N // 512):
                ps = psB.tile([P, 512], f32, tag="ps")
                nc.tensor.matmul(ps[:], W(f"WkT{li}"),
                                 hT[:, 512 * t:512 * (t + 1)], start=True, stop=True)
                nc.vector.tensor_copy(KT[:, 512 * t:512 * (t + 1)], ps[:])
            V = state.tile([P, N // P, P], f32, tag="V")
            for t in range(N // P):
                ps = psB.tile([P, 512], f32, tag="ps")
                nc.tensor.matmul(ps[:, :P], hT[:, P * t:P * (t + 1)],
                                 W(f"WvT{li}"), start=True, stop=True)
                nc.vector.tensor_copy(V[:, t, :], ps[:, :P])

            o_ps = psC.tile([P, 512], f32, tag="o_ps")
            den_ps = psC.tile([P, 512], f32, tag="den_ps")
            nk = N // P
            for kt in range(nk):
                lg = psA.tile([P, NH * 512], f32, tag="lg")
                for h in range(NH):
                    nc.tensor.matmul(
                        lg[:, 512 * h:512 * (h + 1)],
                        KT[DH * h:DH * (h + 1), P * kt:P * (kt + 1)],
                        QT[DH * h:DH * (h + 1), :],
                        start=True, stop=True, tile_position=(DH * h, 0))
                ex = big.tile([P, NH * 512], f32, tag="big")
                nc.scalar.activation(ex[:], lg[:], AF.Exp)
                for h in range(NH):
                    nc.tensor.matmul(
                        o_ps[DH * h:DH * (h + 1), :],
                        V[:, kt, DH * h:DH * (h + 1)],
                        ex[:, 512 * h:512 * (h + 1)],
                        start=(kt == 0), stop=(kt == nk - 1),
                        tile_position=(0, DH * h))
                for h in range(NH):
                    nc.tensor.matmul(
                        den_ps[DH * h:DH * (h + 1), :],
                        ones32[:, :],
                        ex[:, 512 * h:512 * (h + 1)],
                        start=(kt == 0), stop=(kt == nk - 1),
                        tile_position=(0, DH * h))

            rd = state.tile([P, NL], f32, tag="rd")
            nc.vector.reciprocal(rd[:], den_ps[:])
            oT = state.tile([P, NL], f32, tag="oT")
            nc.vector.tensor_tensor(out=oT[:], in0=o_ps[:], in1=rd[:], op=OP.mult)
            ps = psB.tile([P, 512], f32, tag="ps")
            nc.tensor.matmul(ps[:], W(f"WoT{li}"), oT[:], start=True, stop=True)
            h_att = state.tile([P, NL], f32, tag="h_att")
            nc.vector.tensor_tensor(out=h_att[:], in0=ps[:], in1=hTown[:], op=OP.add)

            # ---- combine + FFN
            out0 = state.tile([P, NL], f32, tag="out0")
            nc.vector.tensor_tensor(out=out0[:], in0=h_loc[:], in1=h_att[:], op=OP.add)
            fm = state.tile([P, 2, NL], f32, tag="fm")
            for hh in range(2):
                ps = psB.tile([P, 512], f32, tag="ps")
                nc.tensor.matmul(ps[:], W(f"mW1T{li}")[:, P * hh:P * (hh + 1)],
                                 out0[:], start=True, stop=True)
                nc.vector.tensor_scalar_max(fm[:, hh, :], ps[:], 0.0)
            ps = psB.tile([P, 512], f32, tag="ps")
            nc.tensor.matmul(ps[:], W(f"mW2Ta{li}"), fm[:, 0, :], start=True, stop=False)
            nc.tensor.matmul(ps[:], W(f"mW2Tb{li}"), fm[:, 1, :], start=False, stop=True)
            outT = state.tile([P, NL], f32, tag="outT")
            nc.vector.tensor_tensor(out=outT[:], in0=ps[:], in1=out0[:], op=OP.add)

            # ---- transpose own out to node-major, AllGather
            for w in range(NWIN):
                t128 = tp.tile([P, P], f32, tag="tp")
                trans128(t128[:], outT[:, P * w:P * (w + 1)])
                nc.sync.dma_start(cc_in[li][P * w:P * (w + 1), :], t128[:])
            nc.gpsimd.collective_compute(
                "AllGather", OP.bypass,
                ins=[cc_in[li][:]], outs=[cc_out[li][:]],
                replica_groups=[list(range(NCORES))])

            # ---- graph norm over the full gathered pre-norm output --------
            # stats via ones-matmuls on node-major chunks
            sum_ps = psC.tile([P, 512], f32, tag="o_ps")
            ssq_ps = psC.tile([P, 512], f32, tag="den_ps")
            nchk = N // GROUP
            for g in range(nchk):
                onm = stream.tile([P, gpc, P], f32, tag="gat")
                nc.sync.dma_start(
                    onm[:], cc_out[li][GROUP * g:GROUP * (g + 1), :].rearrange(
                        "(k p) c -> p k c", p=P))
                sq = stream.tile([P, gpc, P], f32, tag="eet")
                nc.vector.tensor_tensor(out=sq[:], in0=onm[:], in1=onm[:],
                                        op=OP.mult)
                for k in range(gpc):
                    t = g * gpc + k
                    nc.tensor.matmul(sum_ps[:1, :P], ones32[:, :1], onm[:, k, :],
                                     start=(t == 0), stop=(t == N // P - 1))
                    nc.tensor.matmul(ssq_ps[:1, :P], ones32[:, :1], sq[:, k, :],
                                     start=(t == 0), stop=(t == N // P - 1))
            # mn = sum/N ; xs = x - a*mn ; var = ssq/N - amn*(2*mn - amn)
            alpha = meta["alpha"][li]
            mn = rows.tile([1, P], f32, tag="mn")
            nc.vector.tensor_scalar_mul(mn[:], sum_ps[:1, :P], 1.0 / N)
            ex2 = rows.tile([1, P], f32, tag="ex2")
            nc.vector.tensor_scalar_mul(ex2[:], ssq_ps[:1, :P], 1.0 / N)
            amn = rows.tile([1, P], f32, tag="amn")
            nc.vector.tensor_scalar_mul(amn[:], mn[:], alpha)
            tmp = rows.tile([1, P], f32, tag="tmp")
            nc.vector.tensor_scalar_mul(tmp[:], mn[:], 2.0)
            nc.vector.tensor_tensor(out=tmp[:], in0=tmp[:], in1=amn[:], op=OP.subtract)
            nc.vector.tensor_tensor(out=tmp[:], in0=tmp[:], in1=amn[:], op=OP.mult)
            var = rows.tile([1, P], f32, tag="var")
            nc.vector.tensor_tensor(out=var[:], in0=ex2[:], in1=tmp[:], op=OP.subtract)
            sd = rows.tile([1, P], f32, tag="sd")
            nc.scalar.activation(sd[:], var[:], AF.Sqrt)
            nc.vector.tensor_scalar_add(sd[:], sd[:], 1e-5)
            rinv = rows.tile([1, P], f32, tag="rinv")
            nc.vector.reciprocal(rinv[:], sd[:])
            # s1 = gamma * rinv ; t2 = beta - s1 * amn
            s1 = rows.tile([1, P], f32, tag="s1")
            nc.vector.tensor_tensor(out=s1[:], in0=rinv[:], in1=W(f"gnG{li}"),
                                    op=OP.mult)
            t2 = rows.tile([1, P], f32, tag="t2")
            nc.vector.tensor_tensor(out=t2[:], in0=s1[:], in1=amn[:], op=OP.mult)
            nc.vector.tensor_tensor(out=t2[:], in0=W(f"gnB{li}"), in1=t2[:],
                                    op=OP.subtract)
            # broadcast coeff rows across partitions via PE
            bs_ps = psB.tile([P, 512], f32, tag="ps")
            nc.tensor.matmul(bs_ps[:, :P], ones_row[:], s1[:], start=True, stop=True)
            bs1 = state.tile([P, P], f32, tag="bs1")
            nc.vector.tensor_copy(bs1[:], bs_ps[:, :P])
            bt_ps = psB.tile([P, 512], f32, tag="ps")
            nc.tensor.matmul(bt_ps[:, :P], ones_row[:], t2[:], start=True, stop=True)
            bt2 = state.tile([P, P], f32, tag="bt2")
            nc.vector.tensor_copy(bt2[:], bt_ps[:, :P])

            # normalize (second pass) -> h node-major -> DRAM + hT
            hT_new = hpool.tile([P, N], f32, tag="hT")
            for g in range(nchk):
                onm = stream.tile([P, gpc, P], f32, tag="gat")
                nc.sync.dma_start(
                    onm[:], cc_out[li][GROUP * g:GROUP * (g + 1), :].rearrange(
                        "(k p) c -> p k c", p=P))
                hnm_t = stream.tile([P, gpc, P], f32, tag="eet")
                nc.vector.tensor_tensor(
                    out=hnm_t[:], in0=onm[:],
                    in1=bs1[:, None, :].to_broadcast([P, gpc, P]), op=OP.mult)
                nc.vector.tensor_tensor(
                    out=hnm_t[:], in0=hnm_t[:],
                    in1=bt2[:, None, :].to_broadcast([P, gpc, P]), op=OP.add)
                nc.sync.dma_start(
                    hnm_d[li + 1][GROUP * g:GROUP * (g + 1), :].rearrange(
                        "(k p) c -> p k c", p=P), hnm_t[:])
                for k in range(gpc):
                    t = g * gpc + k
                    trans128(hT_new[:, P * t:P * (t + 1)], hnm_t[:, k, :])
            hT = hT_new

        # ---------------- decoder: od = h_own @ (decW @ h^T) ---------------
        zT = state.tile([P, N], f32, tag="KT")
        for t in range(N // 512):
            ps = psB.tile([P, 512], f32, tag="ps")
            nc.tensor.matmul(ps[:], W("decWT"), hT[:, 512 * t:512 * (t + 1)],
                             start=True, stop=True)
            nc.vector.tensor_copy(zT[:, 512 * t:512 * (t + 1)], ps[:])
        hnm_own = state.tile([P, NWIN, P], f32, tag="hnm_own")
        nc.gpsimd.dma_gather(
            out_ap=hnm_own[:], in_ap=hnm_d[L][:], idxs_ap=ownidx_s[:],
            num_idxs=NL, num_idxs_reg=NL, elem_size=HD)
        hTown = state.tile([P, NL], f32, tag="hTown")
        for w in range(NWIN):
            trans128(hTown[:, P * w:P * (w + 1)], hnm_own[:, w, :])
        for qt in range(NL // P):
            for half in range(2):
                ost = big.tile([P, 2048], f32, tag="big")
                for t in range(4):
                    c0 = 2048 * half + 512 * t
                    ps = psB.tile([P, 512], f32, tag="ps")
                    nc.tensor.matmul(ps[:], hTown[:, P * qt:P * (qt + 1)],
                                     zT[:, c0:c0 + 512], start=True, stop=True)
                    nc.vector.tensor_copy(ost[:, 512 * t:512 * (t + 1)], ps[:])
                nc.sync.dma_start(
                    od_d[P * qt:P * (qt + 1), 2048 * half:2048 * (half + 1)],
                    ost[:])

        for pl in reversed((wpool, hpool, state, stream, big, tp, rows, psA, psB, psC)):
            pl.release()

    nc.compile()
    return nc


# ---------------------------------------------------------------- entry
def kernel(x, edge_attr, params, edge_index, trace=False):
    global LAST_RESULTS
    in_maps, meta = _prep(x, edge_attr, params, edge_index)
    nc = _build(meta)
    res = run_bass_kernel_spmd(nc, in_maps, core_ids=list(range(NCORES)),
                               trace=trace)
    LAST_RESULTS = res
    out = np.concatenate([res.results[c]["od"] for c in range(NCORES)], axis=0)
    return out


# revision 17
# speedup vs baseline: 1.3673x; 1.3673x over previous
"""Trainium2 Bass kernel for nn_GPSODModel (GraphGPS: GINEConv + global MHA + FFN +
GraphNorm x3, bilinear O-D pair decoder).

Self-contained: takes FULL inputs, shards across 8 NeuronCores internally
(nodes + their incident edges by destination), runs one SPMD Bass program with
one AllGather per layer, returns the FULL [N, N] output.

Layout: activations feature-major on chip ("xT": features on the 128 SBUF
partitions, nodes on the free axis); node-major copies in DRAM for the per-edge
source gather (dma_gather).  f32 residual stream; bf16 for the high-volume
matmul operands (attention Q/K/V/exp, edge messages + one-hot scatter).
Softmax denominators ride as an appended ones-column in V (head h occupies a
64-row strip: 32 V dims, 1 ones col, 31 zeros), then a PE selector matmul
broadcasts the reciprocal back over the head's rows.
"""

import sys

sys.path.insert(0, "/opt/trn_rl_repo")

import numpy as np
import ml_dtypes

import concourse.bacc as bacc
import concourse.bass as bass
import concourse.mybir as mybir
import concourse.tile as tile
from concourse.masks import make_identity
from concourse.bass_utils import run_bass_kernel_spmd

# ---------------------------------------------------------------- constants
N, E = 4096, 131072
IDIM, ED = 64, 32
HD, L, NH = 128, 3, 4
DH = HD // NH
SCALE = 1.0 / float(np.sqrt(DH))

NCORES = 8
NL = N // NCORES            # own nodes per core (512)
NWIN = NL // 128            # dst windows per core (4)
P = 128
GROUP = 1024                # edges per gather/stream group (8 chunks of 128)

f32 = mybir.dt.float32
bf16 = mybir.dt.bfloat16
i16 = mybir.dt.int16

AF = mybir.ActivationFunctionType
OP = mybir.AluOpType

LAST_RESULTS = None         # test harness reads timing info from here
import os
STATS_CAST_DMA = os.environ.get("STATS_CAST_DMA", "1") == "1"
QK_F32 = os.environ.get("QK_F32", "0") == "1"
USE_TRANS512 = os.environ.get("USE_TRANS512", "1") == "1"



# ---------------------------------------------------------------- host prep
class _WPack:
    """Packs host weight matrices into one [128, WC] blob."""

    def __init__(self, dtype=np.float32):
        self.cols = 0
        self.entries = {}       # name -> (rows, c0, ncols)
        self.mats = {}
        self.dtype = dtype

    def add(self, name, mat):
        mat = np.asarray(mat, np.float32)
        assert mat.ndim == 2 and mat.shape[0] <= P, (name, mat.shape)
        r, c = mat.shape
        self.entries[name] = (r, self.cols, c)
        self.mats[name] = mat
        self.cols += c

    def blob(self):
        out = np.zeros((P, self.cols), np.float32)
        for name, (r, c0, c) in self.entries.items():
            out[:r, c0:c0 + c] = self.mats[name]
        return out.astype(self.dtype)


def _pack_weights(p):
    w = _WPack()
    T = lambda m: np.ascontiguousarray(np.asarray(m, np.float32).T)
    w.add("npW1T", T(p["np_W1"]))            # [64, 128]
    w.add("npW2T", T(p["np_W2"]))            # [128, 128]
    w.add("epW1T", T(p["ep_W1"]))            # [32, 128]
    for i in range(L):
        w.add(f"gW1T{i}", T(p["gine_W1"][i]))
        w.add(f"gW2T{i}", T(p["gine_W2"][i]))
        wo = np.asarray(p["attn_Wo"][i], np.float32)   # [HD, HD]
        for h in range(NH):
            w.add(f"WoTh{i}_{h}", np.ascontiguousarray(
                wo[:, DH * h:DH * (h + 1)].T))
        w.add(f"mW1T{i}", T(p["mlp_W1"][i]))         # [128, 256]
        m2t = T(p["mlp_W2"][i])                      # [256, 128]
        w.add(f"mW2Ta{i}", m2t[0:P])
        w.add(f"mW2Tb{i}", m2t[P:2 * P])
        w.add(f"gnG{i}", np.asarray(p["gn_gamma"][i], np.float32)[None, :])
        w.add(f"gnB{i}", np.asarray(p["gn_beta"][i], np.float32)[None, :])
    w.add("decWT", T(p["dec_W"]))

    w16 = _WPack(dtype=ml_dtypes.bfloat16)
    w16.add("ep2b", T(p["ep_W2"]))
    for i in range(L):
        qkv = np.asarray(p["attn_Wqkv"][i], np.float32)
        w16.add(f"Wq16{i}", T(qkv[0:HD]) * SCALE)
        w16.add(f"Wk16{i}", T(qkv[HD:2 * HD]))
        w16.add(f"Wv16{i}", T(qkv[2 * HD:3 * HD]))
    return w, w16


def _wrap_idx(idx, cols):
    """dma_gather index layout: elem i read from [i%16, i//16]; replicate to
    all 128 partitions."""
    n = idx.shape[0]
    assert n % 16 == 0
    a = np.zeros((16, cols), np.int16)
    a[:, : n // 16] = np.asarray(idx, np.int16).reshape(-1, 16).T
    return np.tile(a, (8, 1))


def _prep(x, edge_attr, params, edge_index):
    x = np.asarray(x, np.float32)
    edge_attr = np.asarray(edge_attr, np.float32)
    edge_index = np.asarray(edge_index)
    src, dst = edge_index[0].astype(np.int64), edge_index[1].astype(np.int64)

    for k in ("np_b1", "np_b2", "ep_b1", "ep_b2", "gine_b1", "gine_b2",
              "attn_bqkv", "attn_bo", "mlp_b1", "mlp_b2"):
        assert np.abs(np.asarray(params[k])).max() == 0.0, \
            f"nonzero bias {k} unsupported"

    # --- edge sharding: core = dst // NL, window = (dst % NL) // 128
    order = np.lexsort((src, dst))
    ds, ss = dst[order], src[order]
    win_of = ds // 128                       # global window id 0..31
    counts = np.bincount(win_of, minlength=N // 128)
    cpw = counts.reshape(NCORES, NWIN)       # [core, window] edge counts
    M = [int(np.ceil(cpw[:, w].max() / 128)) for w in range(NWIN)]
    g = GROUP // 128
    M[NWIN - 1] += (-sum(M)) % g
    E_pad = 128 * sum(M)

    chunk_window = []
    for w in range(NWIN):
        chunk_window += [w] * M[w]

    wp, wp16 = _pack_weights(params)
    wblob = wp.blob()
    wblob16 = wp16.blob()
    xT = np.ascontiguousarray(x.T)                      # [64, 4096]

    starts = np.zeros(N // 128 + 1, np.int64)
    starts[1:] = np.cumsum(counts)
    woff = np.cumsum([0] + M)                            # chunk offset per window

    in_maps = []
    for c in range(NCORES):
        srcs = np.zeros(E_pad, np.int64)
        dloc = np.full(E_pad, -1, np.int64)             # window-local dst or -1
        ea = np.zeros((E_pad, ED), np.float32)
        for w in range(NWIN):
            gw = c * NWIN + w
            s, e = starts[gw], starts[gw + 1]
            k = e - s
            off = 128 * woff[w]
            srcs[off:off + k] = ss[s:e]
            dloc[off:off + k] = ds[s:e] - c * NL - w * 128
            ea[off:off + k] = edge_attr[order[s:e]]
        oh = np.zeros((E_pad, P), np.float32)
        valid = dloc >= 0
        oh[np.arange(E_pad)[valid], dloc[valid]] = 1.0
        # SBUF layout [p, chunk*128 + d] = oh[chunk*128 + p, d]
        ohs = np.ascontiguousarray(
            oh.reshape(E_pad // P, P, P).transpose(1, 0, 2).reshape(P, E_pad)
        ).astype(ml_dtypes.bfloat16)
        eaT = np.ascontiguousarray(ea.T)                # [32, E_pad]
        srcidx = _wrap_idx(srcs, E_pad // 16)
        ownidx = _wrap_idx(np.arange(c * NL, (c + 1) * NL), NL // 16)
        in_maps.append({
            "xT": xT,
            "eaT": eaT,
            "oh": ohs,
            "srcidx": srcidx,
            "ownidx": ownidx,
            "wblob": wblob,
            "wblob16": wblob16,
        })
    meta = {"E_pad": E_pad, "M": M, "chunk_window": chunk_window,
            "wp": wp, "wp16": wp16,
            "alpha": [float(a) for a in np.asarray(params["gn_alpha"], np.float32)]}
    return in_maps, meta


# ---------------------------------------------------------------- program
def _build(meta):
    E_pad = meta["E_pad"]
    cw = meta["chunk_window"]
    ngroups = E_pad // GROUP
    gpc = GROUP // 128            # chunks per group (16)

    nc = bacc.Bacc(None)

    # --- dram tensors
    xT_d = nc.dram_tensor("xT", [IDIM, N], f32, kind="ExternalInput")
    eaT_d = nc.dram_tensor("eaT", [ED, E_pad], f32, kind="ExternalInput")
    oh_d = nc.dram_tensor("oh", [P, E_pad], bf16, kind="ExternalInput")
    srcidx_d = nc.dram_tensor("srcidx", [P, E_pad // 16], i16, kind="ExternalInput")
    ownidx_d = nc.dram_tensor("ownidx", [P, NL // 16], i16, kind="ExternalInput")
    wblob_d = nc.dram_tensor("wblob", [P, meta["wp"].cols], f32, kind="ExternalInput")
    wblob16_d = nc.dram_tensor("wblob16", [P, meta["wp16"].cols], bf16,
                               kind="ExternalInput")
    od_d = nc.dram_tensor("od", [NL, N], f32, kind="ExternalOutput")

    hnm_d = [nc.dram_tensor(f"hnm{i}", [N, HD], f32) for i in range(L + 1)]
    ee_d = nc.dram_tensor("ee", [E_pad, HD], bf16)
    cc_in = [nc.dram_tensor(f"ccin{i}", [NL, HD], f32) for i in range(L)]
    cc_out = [nc.dram_tensor(f"ccout{i}", [N, HD], f32, addr_space="Shared")
              for i in range(L)]

    with tile.TileContext(nc) as tc:
        wpool = tc.alloc_tile_pool(name="wpool", bufs=1)
        hpool = tc.alloc_tile_pool(name="hpool", bufs=1)
        state = tc.alloc_tile_pool(name="state", bufs=1)
        stream = tc.alloc_tile_pool(name="stream", bufs=2)
        big = tc.alloc_tile_pool(name="big", bufs=2)
        tp = tc.alloc_tile_pool(name="tp", bufs=2)
        rows = tc.alloc_tile_pool(name="rows", bufs=1)
        psA = tc.alloc_tile_pool(name="psA", bufs=1, space="PSUM")
        psB = tc.alloc_tile_pool(name="psB", bufs=2, space="PSUM")
        psC = tc.alloc_tile_pool(name="psC", bufs=1, space="PSUM")

        # --- constants
        wblob = wpool.tile([P, meta["wp"].cols], f32)
        nc.sync.dma_start(wblob[:], wblob_d[:])
        wblob16 = wpool.tile([P, meta["wp16"].cols], bf16)
        nc.sync.dma_start(wblob16[:], wblob16_d[:])

        def W(name):
            r, c0, ncol = meta["wp"].entries[name]
            return wblob[:r, c0:c0 + ncol]

        def W16(name):
            r, c0, ncol = meta["wp16"].entries[name]
            return wblob16[:r, c0:c0 + ncol]

        ident = wpool.tile([P, P], f32)
        make_identity(nc, ident[:])
        ones32b = wpool.tile([P, 32], bf16)
        nc.vector.memset(ones32b[:], 1.0)
        ones_row = wpool.tile([1, P], f32)
        nc.vector.memset(ones_row[:], 1.0)

        srcidx_s = wpool.tile([P, E_pad // 16], i16)
        nc.sync.dma_start(srcidx_s[:], srcidx_d[:])
        ownidx_s = wpool.tile([P, NL // 16], i16)
        nc.sync.dma_start(ownidx_s[:], ownidx_d[:])

        def trans128(dst_ap, src_ap):
            """dst[c, n] = src[n, c] via PE transpose (both [128, 128])."""
            ps = psB.tile([P, 512], f32, tag="ps")
            nc.tensor.transpose(ps[:, :P], src_ap, ident[:])
            nc.vector.tensor_copy(dst_ap, ps[:, :P])

        def trans512(dst_ap, srcs):
            """4x [128,128] transposes -> one [128, 512] copy into dst."""
            if not USE_TRANS512:
                for j, s in enumerate(srcs):
                    trans128(dst_ap[:, P * j:P * (j + 1)], s)
                return
            ps = psB.tile([P, 512], f32, tag="ps")
            for j, s in enumerate(srcs):
                nc.tensor.transpose(ps[:, P * j:P * (j + 1)], s, ident[:])
            nc.vector.tensor_copy(dst_ap, ps[:])

        # ---------------- prologue: h0 = MLP(x), both layouts -------------
        hT = hpool.tile([P, N], f32, tag="hT")
        for t in range(N // 512):
            ps = psB.tile([P, 512], f32, tag="ps")
            xs = stream.tile([IDIM, 512], f32, tag="xs")
            nc.sync.dma_start(xs[:], xT_d[:, 512 * t:512 * (t + 1)])
            nc.tensor.matmul(ps[:], W("npW1T"), xs[:], start=True, stop=True)
            mid = stream.tile([P, 512], f32, tag="xs")
            nc.vector.tensor_scalar_max(mid[:], ps[:], 0.0)
            ps2 = psB.tile([P, 512], f32, tag="ps")
            nc.tensor.matmul(ps2[:], W("npW2T"), mid[:], start=True, stop=True)
            nc.vector.tensor_copy(hT[:, 512 * t:512 * (t + 1)], ps2[:])
            # node-major h0 -> DRAM (for edge gather), 4 tiles of 128
            stg = tp.tile([P, 4, P], f32, tag="tp4")
            trans512(stg[:].rearrange("p k c -> p (k c)"),
                     [hT[:, 512 * t + P * k: 512 * t + P * (k + 1)]
                      for k in range(4)])
            nc.sync.dma_start(
                hnm_d[0][512 * t:512 * (t + 1), :].rearrange(
                    "(k p) c -> p k c", p=P), stg[:])

        # ---------------- prologue: ee = MLP(edge_attr) -> DRAM edge-major
        for g in range(E_pad // 512):
            eas = stream.tile([ED, 512], f32, tag="eas")
            nc.sync.dma_start(eas[:], eaT_d[:, 512 * g:512 * (g + 1)])
            ps = psB.tile([P, 512], f32, tag="ps")
            nc.tensor.matmul(ps[:], W("epW1T"), eas[:], start=True, stop=True)
            mid16 = stream.tile([P, 512], bf16, tag="mid16")
            nc.vector.tensor_scalar_max(mid16[:], ps[:], 0.0)
            # edge-major out: lhsT = mid chunk [c, e], rhs = ep2b [c, c2]
            ps2 = psB.tile([P, 512], f32, tag="ps")
            for k in range(4):
                nc.tensor.matmul(ps2[:, P * k:P * (k + 1)],
                                 mid16[:, P * k:P * (k + 1)],
                                 W16("ep2b"), start=True, stop=True)
            stg = tp.tile([P, 4, P], bf16, tag="tp4b")
            nc.vector.tensor_copy(stg[:].rearrange("p k c -> p (k c)"), ps2[:])
            nc.sync.dma_start(
                ee_d[512 * g:512 * (g + 1), :].rearrange(
                    "(k p) c -> p k c", p=P), stg[:])

        # ---------------- layers ------------------------------------------
        for li in range(L):
            # own node rows (node-major) for this layer
            hnm_own = state.tile([P, NWIN, P], f32, tag="hnm_own")
            nc.gpsimd.dma_gather(
                out_ap=hnm_own[:], in_ap=hnm_d[li][:], idxs_ap=ownidx_s[:],
                num_idxs=NL, num_idxs_reg=NL, elem_size=HD)
            hTown = state.tile([P, NL], f32, tag="hTown")
            trans512(hTown[:], [hnm_own[:, w, :] for w in range(NWIN)])
            hTownb = state.tile([P, NL], bf16, tag="hTownb")
            nc.vector.tensor_copy(hTownb[:], hTown[:])
            # bf16 copy of full h (feature-major) for K/V production
            hTb = state.tile([P, N], bf16, tag="hTb")
            for t in range(N // 512):
                nc.vector.tensor_copy(hTb[:, 512 * t:512 * (t + 1)],
                                      hT[:, 512 * t:512 * (t + 1)])

            # ---- GINE: gather h[src], msg = relu(h_src + ee), scatter-add
            hgaT = state.tile([P, NL], f32, tag="hgaT")
            aggr_ps = None
            cur_w = -1
            left = {w: meta["M"][w] for w in range(NWIN)}
            for g in range(ngroups):
                gat = stream.tile([P, gpc, P], f32, tag="gat")
                nc.gpsimd.dma_gather(
                    out_ap=gat[:], in_ap=hnm_d[li][:],
                    idxs_ap=srcidx_s[:, (GROUP // 16) * g:(GROUP // 16) * (g + 1)],
                    num_idxs=GROUP, num_idxs_reg=GROUP, elem_size=HD)
                eet = stream.tile([P, gpc, P], bf16, tag="eet")
                nc.sync.dma_start(
                    eet[:],
                    ee_d[GROUP * g:GROUP * (g + 1), :].rearrange(
                        "(k p) c -> p k c", p=P))
                oht = stream.tile([P, GROUP], bf16, tag="oht")
                nc.sync.dma_start(oht[:], oh_d[:, GROUP * g:GROUP * (g + 1)])
                nc.vector.tensor_tensor(out=gat[:], in0=gat[:], in1=eet[:],
                                        op=OP.add)
                msg = stream.tile([P, gpc, P], bf16, tag="msg")
                nc.vector.tensor_scalar_max(msg[:], gat[:], 0.0)
                for k in range(gpc):
                    chunk = g * gpc + k
                    w = cw[chunk]
                    if w != cur_w:
                        aggr_ps = psB.tile([P, 512], f32, tag="ps")
                        cur_w = w
                    first = left[w] == meta["M"][w]
                    left[w] -= 1
                    last = left[w] == 0
                    nc.tensor.matmul(
                        aggr_ps[:, :P],
                        oht[:, P * k:P * (k + 1)],
                        msg[:, k, :],
                        start=first, stop=last)
                    if last:
                        # h + aggr (node-major), transpose into hgaT
                        hga = tp.tile([P, P], f32, tag="tp")
                        nc.vector.tensor_tensor(
                            out=hga[:], in0=aggr_ps[:, :P],
                            in1=hnm_own[:, w, :], op=OP.add)
                        trans128(hgaT[:, P * w:P * (w + 1)], hga[:])

            # GINE MLP + residual
            ps = psB.tile([P, 512], f32, tag="ps")
            nc.tensor.matmul(ps[:], W(f"gW1T{li}"), hgaT[:], start=True, stop=True)
            gmid = state.tile([P, NL], f32, tag="gmid")
            nc.vector.tensor_scalar_max(gmid[:], ps[:], 0.0)
            ps2 = psB.tile([P, 512], f32, tag="ps")
            nc.tensor.matmul(ps2[:], W(f"gW2T{li}"), gmid[:], start=True, stop=True)
            h_loc = state.tile([P, NL], f32, tag="h_loc")
            nc.vector.tensor_tensor(out=h_loc[:], in0=ps2[:], in1=hTown[:], op=OP.add)

            # ---- attention: QT own, KT/V all nodes (bf16)
            qk_dt = f32 if QK_F32 else bf16
            QT = state.tile([P, NL], qk_dt, tag="QT")
            ps = psB.tile([P, 512], f32, tag="ps")
            if QK_F32:
                nc.tensor.matmul(ps[:], W16(f"Wq16{li}"), hTownb[:],
                                 start=True, stop=True)
            else:
                nc.tensor.matmul(ps[:], W16(f"Wq16{li}"), hTownb[:],
                                 start=True, stop=True)
            nc.vector.tensor_copy(QT[:], ps[:])
            KT = state.tile([P, N], qk_dt, tag="KT")
            for t in range(N // 512):
                ps = psB.tile([P, 512], f32, tag="ps")
                nc.tensor.matmul(ps[:], W16(f"Wk16{li}"),
                                 hTb[:, 512 * t:512 * (t + 1)], start=True, stop=True)
                nc.vector.tensor_copy(KT[:, 512 * t:512 * (t + 1)], ps[:])
            # V2: per head a 64-col strip [V_h | ones | zeros]
            nk = N // P
            V2 = state.tile([P, nk, NH * 33 + 31], bf16, tag="V2")
            nc.vector.memset(V2[:], 0.0)
            nc.vector.memset(
                V2[:, :, 0:NH * 33].rearrange(
                    "p k (h d) -> p k h d", h=NH)[:, :, :, DH:DH + 1],
                1.0)
            for t4 in range(nk // 4):
                ps = psB.tile([P, 512], f32, tag="ps")
                for j in range(4):
                    t = 4 * t4 + j
                    nc.tensor.matmul(ps[:, P * j:P * (j + 1)],
                                     hTb[:, P * t:P * (t + 1)],
                                     W16(f"Wv16{li}"), start=True, stop=True)
                nc.vector.tensor_copy(
                    V2[:, 4 * t4:4 * (t4 + 1), 0:NH * 33].rearrange(
                        "p j (h d) -> p j h d", h=NH)[:, :, :, 0:DH],
                    ps[:].rearrange("p (j h d) -> p j h d", j=4, h=NH))

            o_ps = [psC.tile([64, 512], f32, tag=f"o{b}", name=f"o_ps{b}")
                    for b in range(NH)]
            for kt in range(nk):
                for half in range(2):
                    lg = psA.tile([P, 2 * 512], f32, tag="lg")
                    for j in range(2):
                        h = 2 * half + j
                        nc.tensor.matmul(
                            lg[:, 512 * j:512 * (j + 1)],
                            KT[DH * h:DH * (h + 1), P * kt:P * (kt + 1)],
                            QT[DH * h:DH * (h + 1), :],
                            start=True, stop=True, tile_position=(DH * h, 0))
                    ex = big.tile([P, 2 * 512], bf16, tag="exp")
                    nc.scalar.activation(ex[:], lg[:], AF.Exp)
                    for j in range(2):
                        h = 2 * half + j
                        nc.tensor.matmul(
                            o_ps[h][0:64, :],
                            V2[:, kt, 33 * h:33 * h + 64],
                            ex[:, 512 * j:512 * (j + 1)],
                            start=(kt == 0), stop=(kt == nk - 1))

            # normalize: denom sits at row 32 of each o half-bank
            ps = psB.tile([P, 512], f32, tag="ps")
            for h in range(NH):
                rdh = rows.tile([1, 512], f32, tag="rdh")
                nc.vector.reciprocal(rdh[:], o_ps[h][32:33, :])
                rb_ps = psB.tile([P, 512], f32, tag="ps")
                nc.tensor.matmul(rb_ps[0:DH, :], ones_row[:1, 0:DH], rdh[:],
                                 start=True, stop=True)
                rb = state.tile([DH, 512], f32, tag="rb")
                nc.vector.tensor_copy(rb[:], rb_ps[0:DH, :])
                ot = state.tile([DH, 512], f32, tag=f"oT{h}")
                nc.vector.tensor_tensor(out=ot[:], in0=o_ps[h][0:DH, :],
                                        in1=rb[:], op=OP.mult)
                nc.tensor.matmul(ps[:], W(f"WoTh{li}_{h}"), ot[:],
                                 start=(h == 0), stop=(h == NH - 1))
            h_att = state.tile([P, NL], f32, tag="h_att")
            nc.vector.tensor_tensor(out=h_att[:], in0=ps[:], in1=hTown[:], op=OP.add)

            # ---- combine + FFN
            out0 = state.tile([P, NL], f32, tag="out0")
            nc.vector.tensor_tensor(out=out0[:], in0=h_loc[:], in1=h_att[:], op=OP.add)
            fm = state.tile([P, 2, NL], f32, tag="fm")
            for hh in range(2):
                ps = psB.tile([P, 512], f32, tag="ps")
                nc.tensor.matmul(ps[:], W(f"mW1T{li}")[:, P * hh:P * (hh + 1)],
                                 out0[:], start=True, stop=True)
                nc.vector.tensor_scalar_max(fm[:, hh, :], ps[:], 0.0)
            ps = psB.tile([P, 512], f32, tag="ps")
            nc.tensor.matmul(ps[:], W(f"mW2Ta{li}"), fm[:, 0, :], start=True, stop=False)
            nc.tensor.matmul(ps[:], W(f"mW2Tb{li}"), fm[:, 1, :], start=False, stop=True)
            outT = state.tile([P, NL], f32, tag="outT")
            nc.vector.tensor_tensor(out=outT[:], in0=ps[:], in1=out0[:], op=OP.add)

            # ---- transpose own out to node-major, AllGather
            stg = tp.tile([P, 4, P], f32, tag="tp4")
            trans512(stg[:].rearrange("p k c -> p (k c)"),
                     [outT[:, P * w:P * (w + 1)] for w in range(NWIN)])
            nc.sync.dma_start(
                cc_in[li][:].rearrange("(k p) c -> p k c", p=P), stg[:])
            nc.gpsimd.collective_compute(
                "AllGather", OP.bypass,
                ins=[cc_in[li][:]], outs=[cc_out[li][:]],
                replica_groups=[list(range(NCORES))])

            # ---- graph norm stats (bf16 cast-load + ones matmuls) ---------
            sum_ps = psC.tile([64, 512], f32, tag="o0")
            ssq_ps = psC.tile([64, 512], f32, tag="o1")
            nchk = N // GROUP
            nt = (N // P) // 4
            for g in range(nchk):
                onmb = stream.tile([P, gpc, P], bf16, tag="oht")
                if STATS_CAST_DMA:
                    nc.gpsimd.dma_start(
                        onmb[:],
                        cc_out[li][GROUP * g:GROUP * (g + 1), :].rearrange(
                            "(k p) c -> p k c", p=P))
                else:
                    onmf = stream.tile([P, gpc, P], f32, tag="gat")
                    nc.sync.dma_start(
                        onmf[:],
                        cc_out[li][GROUP * g:GROUP * (g + 1), :].rearrange(
                            "(k p) c -> p k c", p=P))
                    nc.vector.tensor_copy(onmb[:], onmf[:])
                sq = stream.tile([P, gpc, P], bf16, tag="eet")
                nc.vector.tensor_tensor(out=sq[:], in0=onmb[:], in1=onmb[:],
                                        op=OP.mult)
                for q in range(gpc // 4):
                    t = g * (gpc // 4) + q
                    nc.tensor.matmul(
                        sum_ps[:1, :], ones32b[:, :1],
                        onmb[:, 4 * q:4 * (q + 1), :].rearrange("p k c -> p (k c)"),
                        start=(t == 0), stop=(t == nt - 1))
                    nc.tensor.matmul(
                        ssq_ps[:1, :], ones32b[:, :1],
                        sq[:, 4 * q:4 * (q + 1), :].rearrange("p k c -> p (k c)"),
                        start=(t == 0), stop=(t == nt - 1))
            # fold the 4 sub-sums: view [1, 128(step1) x 4(step128)], reduce X
            sumr = rows.tile([1, 512], f32, tag="sumr")
            nc.vector.tensor_copy(sumr[:], sum_ps[:1, :])
            ssqr = rows.tile([1, 512], f32, tag="ssqr")
            nc.vector.tensor_copy(ssqr[:], ssq_ps[:1, :])
            mn = rows.tile([1, P], f32, tag="mn")
            nc.vector.tensor_reduce(
                mn[:], sumr[:].rearrange("a (k c) -> a c k", k=4),
                axis=mybir.AxisListType.X, op=OP.add)
            ex2 = rows.tile([1, P], f32, tag="ex2")
            nc.vector.tensor_reduce(
                ex2[:], ssqr[:].rearrange("a (k c) -> a c k", k=4),
                axis=mybir.AxisListType.X, op=OP.add)
            nc.vector.tensor_scalar_mul(mn[:], mn[:], 1.0 / N)
            nc.vector.tensor_scalar_mul(ex2[:], ex2[:], 1.0 / N)
            # mn = sum/N ; xs = x - a*mn ; var = ex2 - amn*(2*mn - amn)
            alpha = meta["alpha"][li]
            amn = rows.tile([1, P], f32, tag="amn")
            nc.vector.tensor_scalar_mul(amn[:], mn[:], alpha)
            tmp = rows.tile([1, P], f32, tag="tmp")
            nc.vector.tensor_scalar_mul(tmp[:], mn[:], 2.0)
            nc.vector.tensor_tensor(out=tmp[:], in0=tmp[:], in1=amn[:], op=OP.subtract)
            nc.vector.tensor_tensor(out=tmp[:], in0=tmp[:], in1=amn[:], op=OP.mult)
            var = rows.tile([1, P], f32, tag="var")
            nc.vector.tensor_tensor(out=var[:], in0=ex2[:], in1=tmp[:], op=OP.subtract)
            sd = rows.tile([1, P], f32, tag="sd")
            nc.scalar.activation(sd[:], var[:], AF.Sqrt)
            nc.vector.tensor_scalar_add(sd[:], sd[:], 1e-5)
            rinv = rows.tile([1, P], f32, tag="rinv")
            nc.vector.reciprocal(rinv[:], sd[:])
            # s1 = gamma * rinv ; t2 = beta - s1 * amn
            s1 = rows.tile([1, P], f32, tag="s1")
            nc.vector.tensor_tensor(out=s1[:], in0=rinv[:], in1=W(f"gnG{li}"),
                                    op=OP.mult)
            t2 = rows.tile([1, P], f32, tag="t2")
            nc.vector.tensor_tensor(out=t2[:], in0=s1[:], in1=amn[:], op=OP.mult)
            nc.vector.tensor_tensor(out=t2[:], in0=W(f"gnB{li}"), in1=t2[:],
                                    op=OP.subtract)
            # broadcast coeff rows across partitions via PE
            bs_ps = psB.tile([P, 512], f32, tag="ps")
            nc.tensor.matmul(bs_ps[:, :P], ones_row[:], s1[:], start=True, stop=True)
            bs1 = state.tile([P, P], f32, tag="bs1")
            nc.vector.tensor_copy(bs1[:], bs_ps[:, :P])
            bt_ps = psB.tile([P, 512], f32, tag="ps")
            nc.tensor.matmul(bt_ps[:, :P], ones_row[:], t2[:], start=True, stop=True)
            bt2 = state.tile([P, P], f32, tag="bt2")
            nc.vector.tensor_copy(bt2[:], bt_ps[:, :P])

            # normalize (second pass) -> h node-major -> DRAM + hT
            hT_new = hpool.tile([P, N], f32, tag="hT")
            for g in range(nchk):
                onm = stream.tile([P, gpc, P], f32, tag="gat")
                nc.sync.dma_start(
                    onm[:], cc_out[li][GROUP * g:GROUP * (g + 1), :].rearrange(
                        "(k p) c -> p k c", p=P))
                hnm_t = stream.tile([P, gpc, P], f32, tag="hnm_t")
                nc.vector.tensor_tensor(
                    out=hnm_t[:], in0=onm[:],
                    in1=bs1[:, None, :].to_broadcast([P, gpc, P]), op=OP.mult)
                nc.vector.tensor_tensor(
                    out=hnm_t[:], in0=hnm_t[:],
                    in1=bt2[:, None, :].to_broadcast([P, gpc, P]), op=OP.add)
                nc.sync.dma_start(
                    hnm_d[li + 1][GROUP * g:GROUP * (g + 1), :].rearrange(
                        "(k p) c -> p k c", p=P), hnm_t[:])
                for q in range(gpc // 4):
                    t = g * (gpc // 4) + q
                    trans512(hT_new[:, 512 * t:512 * (t + 1)],
                             [hnm_t[:, 4 * q + j, :] for j in range(4)])
            hT = hT_new

        # ---------------- decoder: od = h_own @ (decW @ h^T) ---------------
        zT = state.tile([P, N], f32, tag="hTb")
        for t in range(N // 512):
            ps = psB.tile([P, 512], f32, tag="ps")
            nc.tensor.matmul(ps[:], W("decWT"), hT[:, 512 * t:512 * (t + 1)],
                             start=True, stop=True)
            nc.vector.tensor_copy(zT[:, 512 * t:512 * (t + 1)], ps[:])
        hnm_own = state.tile([P, NWIN, P], f32, tag="hnm_own")
        nc.gpsimd.dma_gather(
            out_ap=hnm_own[:], in_ap=hnm_d[L][:], idxs_ap=ownidx_s[:],
            num_idxs=NL, num_idxs_reg=NL, elem_size=HD)
        hTown = state.tile([P, NL], f32, tag="hTown")
        trans512(hTown[:], [hnm_own[:, w, :] for w in range(NWIN)])
        for qt in range(NL // P):
            for quad in range(4):
                ost = big.tile([P, 1024], f32, tag="exp")
                for t in range(2):
                    c0 = 1024 * quad + 512 * t
                    ps = psB.tile([P, 512], f32, tag="ps")
                    nc.tensor.matmul(ps[:], hTown[:, P * qt:P * (qt + 1)],
                                     zT[:, c0:c0 + 512], start=True, stop=True)
                    nc.vector.tensor_copy(ost[:, 512 * t:512 * (t + 1)], ps[:])
                nc.sync.dma_start(
                    od_d[P * qt:P * (qt + 1), 1024 * quad:1024 * (quad + 1)],
                    ost[:])

        for pl in reversed((wpool, hpool, state, stream, big, tp, rows,
                            psA, psB, psC)):
            pl.release()

    nc.compile()
    return nc


# ---------------------------------------------------------------- entry
def kernel(x, edge_attr, params, edge_index, trace=False):
    global LAST_RESULTS
    in_maps, meta = _prep(x, edge_attr, params, edge_index)
    nc = _build(meta)
    res = run_bass_kernel_spmd(nc, in_maps, core_ids=list(range(NCORES)),
                               trace=trace)
    LAST_RESULTS = res
    out = np.concatenate([res.results[c]["od"] for c in range(NCORES)], axis=0)
    return out
